# revision 20
# baseline (speedup 1.0000x reference)
"""Behler-Parrinello NN potential kernel for 8x Trainium2 NeuronCores.

Strategy (moe_routing — route instead of dense-compute-both-experts):
  - Host: partition atoms by type, pad each per-core type group to a
    multiple of 128, shard across 8 cores (data-parallel over atoms,
    per-type MLP weights replicated). Each core gets a feature-major
    (transposed) [128, n_per_core] fp16 slice of Gs.
  - Device (per core, Bass/Tile), per single-expert block of <=1024
    atoms:
      L1 (normal orientation): psum[hidden_m, atoms] = W1_m^T G, two
        m-pair mega-psums of [128, 2*w]; tanh on the scalar engine in
        [128, 2048] mega-tiles (the +352-cycle ACT overhead amortizes
        2x better than per-m tiles) -> h1 [128 hidden_k, atoms] fp16.
      L2 (FLIPPED): stationary = h1 128-atom column slices, moving =
        W2 k-row blocks [128, 512], accumulated over the 4 k-chunks ->
        psum[atoms, hidden] mega of 4 groups [128, 2048]; tanh mega ->
        h2 [128 atoms, hidden] fp16.  The flip makes the L3 contraction
        a FREE-dim reduction.
      L3 on the VECTOR engine: tensor_tensor_reduce(h2 * w3_replicated,
        add-reduce along free) -> e[atom] fp32.  This removes all M=1
        matmuls from the PE (~17% of its streamed columns) at ~3us/block
        of otherwise-idle DVE time.
    PSUM budget is exactly 8 banks: one live L1 mega (4) + one live L2
    mega (4); the software pipeline interleaves L1(b) with L2(b-1) so
    the PE never waits on an ACT drain.
  - Biases: b1 uses per-m ACT bias APs when nonzero; b2 is injected
    via one extra accumulating matmul per group against a replicated
    b2/128 moving tile when nonzero.  With the all-zero biases of this
    model both paths collapse to mega-ACTs with scalar bias 0.
  - Host: scatter energies back to original atom order, segment-mean
    per molecule with bincount (~0.0001% of the FLOPs).
"""

import os
import sys
import time

sys.path.insert(0, "/opt/trn_rl_repo")

_DBG_NO_TTR = os.environ.get("K_NO_TTR", "0") == "1"
_DBG_ACT1024 = os.environ.get("K_ACT1024", "0") == "1"

import numpy as np

import concourse.bacc as bacc
import concourse.mybir as mybir
from concourse import tile
from concourse.bass_utils import run_bass_kernel_spmd

N_CORES = 8
NUM_GS = 128
HIDDEN = 512
N_MOL = 1024
BLK = 1024           # atoms per block
MCH = HIDDEN // 128  # hidden chunks of 128

F32 = mybir.dt.float32
F16 = mybir.dt.float16
Tanh = mybir.ActivationFunctionType.Tanh
Mult = None  # set lazily from mybir.AluOpType

_PROGRAM_CACHE: dict = {}


def _build_program(n_a: int, n_b: int, zb1: bool, zb2: bool):
    """SPMD Bass program: n_a A-atoms + n_b B-atoms per core (multiples
    of 128). zb1/zb2: biases b1/b2 are all-zero (fast path)."""
    key = (n_a, n_b, zb1, zb2)
    if key in _PROGRAM_CACHE:
        return _PROGRAM_CACHE[key]

    mult = mybir.AluOpType.mult
    add = mybir.AluOpType.add

    ntot = n_a + n_b
    gtot = ntot // 128
    nc = bacc.Bacc("TRN2", target_bir_lowering=False, debug=False,
                   num_devices=N_CORES)

    gst = nc.dram_tensor("gst", [NUM_GS, ntot], F16, kind="ExternalInput")
    e_out = nc.dram_tensor("e_out", [128, gtot], F32, kind="ExternalOutput")
    dram = {}
    for t in ("a", "b"):
        dram[f"w1{t}"] = nc.dram_tensor(f"w1{t}", [NUM_GS, HIDDEN], F16,
                                        kind="ExternalInput")
        dram[f"w2{t}"] = nc.dram_tensor(f"w2{t}", [HIDDEN, HIDDEN], F16,
                                        kind="ExternalInput")
        dram[f"w3{t}"] = nc.dram_tensor(f"w3{t}", [128, HIDDEN], F16,
                                        kind="ExternalInput")
        if not zb1:
            dram[f"b1{t}"] = nc.dram_tensor(f"b1{t}", [128, MCH], F32,
                                            kind="ExternalInput")
        if not zb2:
            dram[f"b2{t}"] = nc.dram_tensor(f"b2{t}", [128, HIDDEN], F16,
                                            kind="ExternalInput")
    if not zb2:
        dram["ones"] = nc.dram_tensor("ones", [128, 128], F16,
                                      kind="ExternalInput")

    # Block schedule: contiguous A atoms then B atoms, single expert per
    # block.  First/last blocks kept small to shorten pipeline fill and
    # drain.
    blocks = []
    off = 0
    for t, n_at in (("a", n_a), ("b", n_b)):
        rem = n_at
        while rem:
            w = min(BLK, rem)
            blocks.append((t, off, w))
            off += w
            rem -= w
    if blocks and blocks[0][2] > 512:
        t0, o0, w0 = blocks[0]
        blocks[0:1] = [(t0, o0, 256), (t0, o0 + 256, w0 - 256)]
    # Taper the drain: halve any wide block among the last three so the
    # post-matmul ACT->product->reduce backlog at exit stays small.
    changed = True
    while changed:
        changed = False
        for j in range(max(0, len(blocks) - 3), len(blocks)):
            tj, oj, wj = blocks[j]
            if wj > 512:
                blocks[j:j + 1] = [(tj, oj, wj // 2),
                                   (tj, oj + wj // 2, wj - wj // 2)]
                changed = True
                break

    first_ex = blocks[0][0]

    with tile.TileContext(nc) as tc:
        with (
            tc.tile_pool(name="wpool", bufs=1) as wpool,
            tc.tile_pool(name="gpool", bufs=4) as gpool,
            tc.tile_pool(name="h1pool", bufs=4) as h1pool,
            tc.tile_pool(name="h2pool", bufs=8) as h2pool,
            tc.tile_pool(name="epool", bufs=6) as epool,
            tc.tile_pool(name="scpool", bufs=8) as scpool,
            tc.tile_pool(name="pl1", bufs=1, space="PSUM") as pl1,
            tc.tile_pool(name="pl2", bufs=2, space="PSUM") as pl2,
        ):
            # Warm the PE (HAM clock gate) with matmuls on scratch SBUF
            # while the first DMAs are in flight; result never read.
            scratch = wpool.tile([128, 512], F16, tag="scratch")
            nc.gpsimd.memset(scratch[:, :], 0)
            wps = pl2.tile([128, 512], F32, tag="l2")
            for i in range(10):
                nc.tensor.matmul(wps[:, :], scratch[:, 0:128], scratch[:, :],
                                 start=(i == 0), stop=(i == 9))

            # Weights: the first expert's w1 leads the sync queue (needed
            # by block 0); everything else rides the gpsimd queue so gs
            # block DMAs (sync) aren't stuck behind weight traffic.
            sb = {}

            def emit_weight_dmas(t, lead_sync):
                w1 = wpool.tile([128, HIDDEN], F16, tag=f"w1{t}")
                eng = nc.sync if lead_sync else nc.gpsimd
                eng.dma_start(w1[:, :], dram[f"w1{t}"][:, :])
                w2 = []
                for k in range(MCH):
                    w2k = wpool.tile([128, HIDDEN], F16, tag=f"w2{t}{k}")
                    nc.gpsimd.dma_start(
                        w2k[:, :], dram[f"w2{t}"][k * 128:(k + 1) * 128, :])
                    w2.append(w2k)
                w3 = wpool.tile([128, HIDDEN], F16, tag=f"w3{t}")
                nc.gpsimd.dma_start(w3[:, :], dram[f"w3{t}"][:, :])
                b1t = None
                if not zb1:
                    b1t = wpool.tile([128, MCH], F32, tag=f"b1{t}")
                    nc.gpsimd.dma_start(b1t[:, :], dram[f"b1{t}"][:, :])
                b2t = None
                if not zb2:
                    b2t = wpool.tile([128, HIDDEN], F16, tag=f"b2{t}")
                    nc.gpsimd.dma_start(b2t[:, :], dram[f"b2{t}"][:, :])
                sb[t] = (w1, w2, w3, b1t, b2t)

            emit_weight_dmas(first_ex, True)
            second_ex = "b" if first_ex == "a" else "a"
            need_second = any(b[0] == second_ex for b in blocks)
            ones_t = None
            if not zb2:
                ones_t = wpool.tile([128, 128], F16, tag="ones")
                nc.gpsimd.dma_start(ones_t[:, :], dram["ones"][:, :])

            gs_of, h1_of, h2cnt, e_of = {}, {}, {}, {}

            def bank_chunks(base, w):
                # chunk [0, w) so that each psum write [base+c0, +cw)
                # stays within one 512-fp32 bank and cw <= 512
                out, c0 = [], 0
                while c0 < w:
                    lim = 512 - ((base + c0) % 512)
                    cw = min(512, w - c0, lim)
                    out.append((c0, cw))
                    c0 += cw
                return out

            def emit_l1(bi, half):
                ex, boff, w = blocks[bi]
                w1, _, _, b1t, _ = sb[ex]
                if half == 0:
                    gs = gpool.tile([128, w], F16, tag="gs")
                    for c0, cw in bank_chunks(0, w):
                        nc.sync.dma_start(gs[:, c0:c0 + cw],
                                          gst[:, boff + c0:boff + c0 + cw])
                    gs_of[bi] = gs
                    h1_of[bi] = h1pool.tile([128, MCH * w], F16, tag="h1", name="h1t")
                gs = gs_of[bi]
                h1t = h1_of[bi]
                ps = pl1.tile([128, 2 * w], F32, tag="l1")
                for ml in range(2):
                    m = 2 * half + ml
                    lhs = w1[:, m * 128:(m + 1) * 128]
                    for c0, cw in bank_chunks(ml * w, w):
                        nc.tensor.matmul(ps[:, ml * w + c0:ml * w + c0 + cw],
                                         lhs, gs[:, c0:c0 + cw],
                                         start=True, stop=True)
                ho = 2 * half * w
                if zb1 and not _DBG_ACT1024:
                    nc.scalar.activation(h1t[:, ho:ho + 2 * w], ps[:, :],
                                         Tanh, bias=0.0, scale=1.0)
                elif zb1:
                    for ml in range(2):
                        m = 2 * half + ml
                        nc.scalar.activation(
                            h1t[:, (m * w):(m + 1) * w],
                            ps[:, ml * w:(ml + 1) * w],
                            Tanh, bias=0.0, scale=1.0)
                else:
                    for ml in range(2):
                        m = 2 * half + ml
                        nc.scalar.activation(
                            h1t[:, (m * w):(m + 1) * w], ps[:, ml * w:(ml + 1) * w],
                            Tanh, bias=b1t[:, m:m + 1], scale=1.0)

            def emit_l2(bi, mega):
                ex, boff, w = blocks[bi]
                _, w2, w3, _, b2t = sb[ex]
                ng = w // 128
                groups = list(range(mega * 2, min(mega * 2 + 2, ng)))
                if not groups:
                    return
                h1t = h1_of[bi]
                ngm = len(groups)
                if bi not in e_of:
                    e_of[bi] = epool.tile([128, ng], F32, tag="e", name="eblk")
                    h2cnt[bi] = 0
                e_blk = e_of[bi]
                ps2 = pl2.tile([128, ngm * 512], F32, tag="l2")
                for gl, g in enumerate(groups):
                    for k in range(MCH):
                        nc.tensor.matmul(
                            ps2[:, gl * 512:(gl + 1) * 512],
                            h1t[:, k * w + g * 128:k * w + (g + 1) * 128],
                            w2[k][:, :],
                            start=(k == 0), stop=(k == MCH - 1) and zb2)
                    if not zb2:
                        nc.tensor.matmul(
                            ps2[:, gl * 512:(gl + 1) * 512],
                            ones_t[:, :], b2t[:, :],
                            start=False, stop=True)
                h2f = h2pool.tile([128, ngm, 512], F16, tag="h2")
                if not _DBG_ACT1024:
                    nc.scalar.activation(h2f[:, :, :], ps2[:, :], Tanh,
                                         bias=0.0, scale=1.0)
                else:
                    for gl in range(ngm):
                        nc.scalar.activation(
                            h2f[:, gl, :],
                            ps2[:, gl * 512:(gl + 1) * 512],
                            Tanh, bias=0.0, scale=1.0)
                # L3: w3-products alternate DVE/gpsimd (both otherwise
                # have slack), segmented 3D add-reduce on DVE
                prod = scpool.tile([128, ngm, 512], F16, tag="prod")
                for gl in range(ngm):
                    eng = nc.vector if gl % 2 == 0 else nc.gpsimd
                    eng.tensor_mul(prod[:, gl, :], h2f[:, gl, :],
                                   w3[:, :])
                nc.vector.tensor_reduce(
                    e_blk[:, groups[0]:groups[0] + ngm], prod[:, :, :],
                    mybir.AxisListType.X, add)
                h2cnt[bi] += ngm
                if h2cnt[bi] == ng:
                    nc.sync.dma_start(
                        e_out[:, boff // 128:boff // 128 + ng], e_blk[:, :])
                    del h1_of[bi], gs_of[bi], e_of[bi]

            # Software pipeline: L1(b) interleaved with L2(b-1) at
            # half-block granularity so PE work alternates between the
            # two 4-bank psum megas and never waits on an ACT drain.
            nblocks = len(blocks)
            for i in range(nblocks + 1):
                cur = i if i < nblocks else None
                prv = i - 1 if 0 <= i - 1 < nblocks else None
                if i == 1 and need_second:
                    # Deferred: the second expert's weights ride the gpsimd
                    # queue AFTER block 0's L3 products, so the early DVE
                    # reduce chain isn't stuck behind ~1MB of weight DMA.
                    emit_weight_dmas(second_ex, False)
                if cur is not None:
                    emit_l1(cur, 0)
                if prv is not None:
                    emit_l2(prv, 0)
                    emit_l2(prv, 1)
                if cur is not None:
                    emit_l1(cur, 1)
                if prv is not None:
                    emit_l2(prv, 2)
                    emit_l2(prv, 3)

    nc.compile()
    _PROGRAM_CACHE[key] = nc
    return nc


def kernel(**inputs) -> np.ndarray:
    Gs = np.ascontiguousarray(np.asarray(inputs["Gs"], dtype=np.float32))
    types = np.asarray(inputs["types"])
    mol_id = np.asarray(inputs["mol_id"])
    n_atoms = Gs.shape[0]

    idx = [np.flatnonzero(types == 0), np.flatnonzero(types != 0)]
    GRAN = 128
    n_a, n_b = (int(-(-len(ix) // (N_CORES * GRAN))) * GRAN for ix in idx)
    npc = n_a + n_b

    GsT = Gs.astype(np.float16).T  # [128, N] fp16 view

    wk = {}
    zb1 = zb2 = True
    for t, pre in (("a", "A"), ("b", "B")):
        wk[f"w1{t}"] = np.ascontiguousarray(
            np.asarray(inputs[f"W1_{pre}"], np.float32).astype(np.float16))
        wk[f"w2{t}"] = np.ascontiguousarray(
            np.asarray(inputs[f"W2_{pre}"], np.float32).astype(np.float16))
        w3col = np.asarray(inputs[f"W3_{pre}"], np.float32)[:, 0]
        wk[f"w3{t}"] = np.ascontiguousarray(
            np.broadcast_to(w3col.astype(np.float16), (128, HIDDEN)))
        b1 = np.asarray(inputs[f"b1_{pre}"], np.float32)
        b2 = np.asarray(inputs[f"b2_{pre}"], np.float32)
        if np.any(b1 != 0):
            zb1 = False
        if np.any(b2 != 0):
            zb2 = False
        wk[f"b1{t}"] = np.ascontiguousarray(b1.reshape(MCH, 128).T)
        wk[f"b2{t}"] = np.ascontiguousarray(np.broadcast_to(
            (b2 / 128.0).astype(np.float16), (128, HIDDEN)))
        wk[f"b3{t}"] = np.float32(
            np.asarray(inputs[f"b3_{pre}"], np.float32).reshape(())
            + np.asarray(inputs[f"off_{pre}"], np.float32).reshape(()))

    base = {f"w{j}{t}": wk[f"w{j}{t}"] for j in (1, 2, 3) for t in ("a", "b")}
    if not zb1:
        base.update({f"b1{t}": wk[f"b1{t}"] for t in ("a", "b")})
    if not zb2:
        base.update({f"b2{t}": wk[f"b2{t}"] for t in ("a", "b")})
        base["ones"] = np.ones((128, 128), np.float16)

    chunks = []  # per core: (a_indices, b_indices)
    in_maps = []
    for i in range(N_CORES):
        ca = idx[0][i * n_a:(i + 1) * n_a]
        cb = idx[1][i * n_b:(i + 1) * n_b]
        chunks.append((ca, cb))
        buf = np.zeros((NUM_GS, npc), np.float16)
        buf[:, :len(ca)] = GsT[:, ca]
        buf[:, n_a:n_a + len(cb)] = GsT[:, cb]
        in_maps.append({"gst": buf, **base})

    nc = _build_program(n_a, n_b, zb1, zb2)
    results = None
    for attempt in range(3):
        try:
            results = run_bass_kernel_spmd(
                nc, in_maps, list(range(N_CORES))).results
            break
        except Exception:
            # Transient NRT/device hiccups usually clear on retry.
            if attempt == 2:
                raise
            time.sleep(2.0)

    e = np.empty(n_atoms, np.float32)
    for i in range(N_CORES):
        r = np.asarray(results[i]["e_out"])  # [128, npc/128]
        flat = r.T.reshape(-1)
        ca, cb = chunks[i]
        e[ca] = flat[:len(ca)] + wk["b3a"]
        e[cb] = flat[n_a:n_a + len(cb)] + wk["b3b"]

    sums = np.bincount(mol_id, weights=e.astype(np.float64),
                       minlength=N_MOL)[:N_MOL]
    counts = np.bincount(mol_id, minlength=N_MOL)[:N_MOL]
    out = sums / np.maximum(counts, 1)
    return out.astype(np.float32)[:, None]


# revision 21
# speedup vs baseline: 1.0266x; 1.0266x over previous
"""Behler-Parrinello NN potential kernel for 8x Trainium2 NeuronCores.

Strategy (moe_routing — route instead of dense-compute-both-experts):
  - Host: partition atoms by type, pad each per-core type group to a
    multiple of 128, shard across 8 cores (data-parallel over atoms,
    per-type MLP weights replicated). Each core gets a feature-major
    (transposed) [128, n_per_core] fp16 slice of Gs.
  - Device (per core, Bass/Tile), per single-expert block of <=1024
    atoms:
      L1 (normal orientation): psum[hidden_m, atoms] = W1_m^T G, two
        m-pair mega-psums of [128, 2*w]; tanh on the scalar engine in
        [128, 2048] mega-tiles (the +352-cycle ACT overhead amortizes
        2x better than per-m tiles) -> h1 [128 hidden_k, atoms] fp16.
      L2 (FLIPPED): stationary = h1 128-atom column slices, moving =
        W2 k-row blocks [128, 512], accumulated over the 4 k-chunks ->
        psum[atoms, hidden] mega of 4 groups [128, 2048]; tanh mega ->
        h2 [128 atoms, hidden] fp16.  The flip makes the L3 contraction
        a FREE-dim reduction.
      L3 on the VECTOR engine: tensor_tensor_reduce(h2 * w3_replicated,
        add-reduce along free) -> e[atom] fp32.  This removes all M=1
        matmuls from the PE (~17% of its streamed columns) at ~3us/block
        of otherwise-idle DVE time.
    PSUM budget is exactly 8 banks: one live L1 mega (4) + one live L2
    mega (4); the software pipeline interleaves L1(b) with L2(b-1) so
    the PE never waits on an ACT drain.
  - Biases: b1 uses per-m ACT bias APs when nonzero; b2 is injected
    via one extra accumulating matmul per group against a replicated
    b2/128 moving tile when nonzero.  With the all-zero biases of this
    model both paths collapse to mega-ACTs with scalar bias 0.
  - Host: scatter energies back to original atom order, segment-mean
    per molecule with bincount (~0.0001% of the FLOPs).
"""

import os
import sys
import time

sys.path.insert(0, "/opt/trn_rl_repo")

_DBG_NO_TTR = os.environ.get("K_NO_TTR", "0") == "1"
_DBG_ACT1024 = os.environ.get("K_ACT1024", "0") == "1"

import numpy as np

import concourse.bacc as bacc
import concourse.mybir as mybir
from concourse import tile
from concourse.bass_utils import run_bass_kernel_spmd

N_CORES = 8
NUM_GS = 128
HIDDEN = 512
N_MOL = 1024
BLK = 1024           # atoms per block
MCH = HIDDEN // 128  # hidden chunks of 128

F32 = mybir.dt.float32
F16 = mybir.dt.float16
Tanh = mybir.ActivationFunctionType.Tanh
Mult = None  # set lazily from mybir.AluOpType

_PROGRAM_CACHE: dict = {}


def _build_program(n_a: int, n_b: int, zb1: bool, zb2: bool):
    """SPMD Bass program: n_a A-atoms + n_b B-atoms per core (multiples
    of 128). zb1/zb2: biases b1/b2 are all-zero (fast path)."""
    key = (n_a, n_b, zb1, zb2)
    if key in _PROGRAM_CACHE:
        return _PROGRAM_CACHE[key]

    mult = mybir.AluOpType.mult
    add = mybir.AluOpType.add

    ntot = n_a + n_b
    gtot = ntot // 128
    nc = bacc.Bacc("TRN2", target_bir_lowering=False, debug=False,
                   num_devices=N_CORES)

    gst = nc.dram_tensor("gst", [NUM_GS, ntot], F16, kind="ExternalInput")
    e_out = nc.dram_tensor("e_out", [128, gtot], F32, kind="ExternalOutput")
    dram = {}
    for t in ("a", "b"):
        dram[f"w1{t}"] = nc.dram_tensor(f"w1{t}", [NUM_GS, HIDDEN], F16,
                                        kind="ExternalInput")
        dram[f"w2{t}"] = nc.dram_tensor(f"w2{t}", [HIDDEN, HIDDEN], F16,
                                        kind="ExternalInput")
        dram[f"w3{t}"] = nc.dram_tensor(f"w3{t}", [128, HIDDEN], F16,
                                        kind="ExternalInput")
        if not zb1:
            dram[f"b1{t}"] = nc.dram_tensor(f"b1{t}", [128, MCH], F32,
                                            kind="ExternalInput")
        if not zb2:
            dram[f"b2{t}"] = nc.dram_tensor(f"b2{t}", [128, HIDDEN], F16,
                                            kind="ExternalInput")
    if not zb2:
        dram["ones"] = nc.dram_tensor("ones", [128, 128], F16,
                                      kind="ExternalInput")

    # Block schedule: contiguous A atoms then B atoms, single expert per
    # block.  First/last blocks kept small to shorten pipeline fill and
    # drain.
    blocks = []
    off = 0
    for t, n_at in (("a", n_a), ("b", n_b)):
        rem = n_at
        while rem:
            w = min(BLK, rem)
            blocks.append((t, off, w))
            off += w
            rem -= w
    if blocks and blocks[0][2] > 512:
        t0, o0, w0 = blocks[0]
        blocks[0:1] = [(t0, o0, 256), (t0, o0 + 256, w0 - 256)]
    if blocks and blocks[-1][2] > 512:
        t1, o1, w1 = blocks[-1]
        blocks[-1:] = [(t1, o1, w1 - 256), (t1, o1 + w1 - 256, 256)]

    first_ex = blocks[0][0]

    with tile.TileContext(nc) as tc:
        with (
            tc.tile_pool(name="wpool", bufs=1) as wpool,
            tc.tile_pool(name="gpool", bufs=4) as gpool,
            tc.tile_pool(name="h1pool", bufs=4) as h1pool,
            tc.tile_pool(name="h2pool", bufs=8) as h2pool,
            tc.tile_pool(name="epool", bufs=6) as epool,
            tc.tile_pool(name="scpool", bufs=8) as scpool,
            tc.tile_pool(name="pl1", bufs=1, space="PSUM") as pl1,
            tc.tile_pool(name="pl2", bufs=2, space="PSUM") as pl2,
        ):
            # Warm the PE (HAM clock gate) with matmuls on scratch SBUF
            # while the first DMAs are in flight; result never read.
            scratch = wpool.tile([128, 512], F16, tag="scratch")
            nc.gpsimd.memset(scratch[:, :], 0)
            wps = pl2.tile([128, 512], F32, tag="l2")
            for i in range(10):
                nc.tensor.matmul(wps[:, :], scratch[:, 0:128], scratch[:, :],
                                 start=(i == 0), stop=(i == 9))

            # Weights: the first expert's w1 leads the sync queue (needed
            # by block 0); everything else rides the gpsimd queue so gs
            # block DMAs (sync) aren't stuck behind weight traffic.
            sb = {}

            def emit_weight_dmas(t, lead_sync):
                w1 = wpool.tile([128, HIDDEN], F16, tag=f"w1{t}")
                eng = nc.sync if lead_sync else nc.gpsimd
                eng.dma_start(w1[:, :], dram[f"w1{t}"][:, :])
                w2 = []
                for k in range(MCH):
                    w2k = wpool.tile([128, HIDDEN], F16, tag=f"w2{t}{k}")
                    nc.gpsimd.dma_start(
                        w2k[:, :], dram[f"w2{t}"][k * 128:(k + 1) * 128, :])
                    w2.append(w2k)
                w3 = wpool.tile([128, HIDDEN], F16, tag=f"w3{t}")
                nc.gpsimd.dma_start(w3[:, :], dram[f"w3{t}"][:, :])
                b1t = None
                if not zb1:
                    b1t = wpool.tile([128, MCH], F32, tag=f"b1{t}")
                    nc.gpsimd.dma_start(b1t[:, :], dram[f"b1{t}"][:, :])
                b2t = None
                if not zb2:
                    b2t = wpool.tile([128, HIDDEN], F16, tag=f"b2{t}")
                    nc.gpsimd.dma_start(b2t[:, :], dram[f"b2{t}"][:, :])
                sb[t] = (w1, w2, w3, b1t, b2t)

            emit_weight_dmas(first_ex, True)
            second_ex = "b" if first_ex == "a" else "a"
            need_second = any(b[0] == second_ex for b in blocks)
            ones_t = None
            if not zb2:
                ones_t = wpool.tile([128, 128], F16, tag="ones")
                nc.gpsimd.dma_start(ones_t[:, :], dram["ones"][:, :])

            gs_of, h1_of, h2cnt, e_of = {}, {}, {}, {}

            def bank_chunks(base, w):
                # chunk [0, w) so that each psum write [base+c0, +cw)
                # stays within one 512-fp32 bank and cw <= 512
                out, c0 = [], 0
                while c0 < w:
                    lim = 512 - ((base + c0) % 512)
                    cw = min(512, w - c0, lim)
                    out.append((c0, cw))
                    c0 += cw
                return out

            def emit_l1(bi, half):
                ex, boff, w = blocks[bi]
                w1, _, _, b1t, _ = sb[ex]
                if half == 0:
                    gs = gpool.tile([128, w], F16, tag="gs")
                    for c0, cw in bank_chunks(0, w):
                        nc.sync.dma_start(gs[:, c0:c0 + cw],
                                          gst[:, boff + c0:boff + c0 + cw])
                    gs_of[bi] = gs
                    h1_of[bi] = h1pool.tile([128, MCH * w], F16, tag="h1", name="h1t")
                gs = gs_of[bi]
                h1t = h1_of[bi]
                ps = pl1.tile([128, 2 * w], F32, tag="l1")
                for ml in range(2):
                    m = 2 * half + ml
                    lhs = w1[:, m * 128:(m + 1) * 128]
                    for c0, cw in bank_chunks(ml * w, w):
                        nc.tensor.matmul(ps[:, ml * w + c0:ml * w + c0 + cw],
                                         lhs, gs[:, c0:c0 + cw],
                                         start=True, stop=True)
                ho = 2 * half * w
                if zb1 and not _DBG_ACT1024:
                    nc.scalar.activation(h1t[:, ho:ho + 2 * w], ps[:, :],
                                         Tanh, bias=0.0, scale=1.0)
                elif zb1:
                    for ml in range(2):
                        m = 2 * half + ml
                        nc.scalar.activation(
                            h1t[:, (m * w):(m + 1) * w],
                            ps[:, ml * w:(ml + 1) * w],
                            Tanh, bias=0.0, scale=1.0)
                else:
                    for ml in range(2):
                        m = 2 * half + ml
                        nc.scalar.activation(
                            h1t[:, (m * w):(m + 1) * w], ps[:, ml * w:(ml + 1) * w],
                            Tanh, bias=b1t[:, m:m + 1], scale=1.0)

            def emit_l2(bi, mega):
                ex, boff, w = blocks[bi]
                _, w2, w3, _, b2t = sb[ex]
                ng = w // 128
                groups = list(range(mega * 2, min(mega * 2 + 2, ng)))
                if not groups:
                    return
                h1t = h1_of[bi]
                ngm = len(groups)
                if bi not in e_of:
                    e_of[bi] = epool.tile([128, ng], F32, tag="e", name="eblk")
                    h2cnt[bi] = 0
                e_blk = e_of[bi]
                ps2 = pl2.tile([128, ngm * 512], F32, tag="l2")
                for gl, g in enumerate(groups):
                    for k in range(MCH):
                        nc.tensor.matmul(
                            ps2[:, gl * 512:(gl + 1) * 512],
                            h1t[:, k * w + g * 128:k * w + (g + 1) * 128],
                            w2[k][:, :],
                            start=(k == 0), stop=(k == MCH - 1) and zb2)
                    if not zb2:
                        nc.tensor.matmul(
                            ps2[:, gl * 512:(gl + 1) * 512],
                            ones_t[:, :], b2t[:, :],
                            start=False, stop=True)
                h2f = h2pool.tile([128, ngm, 512], F16, tag="h2")
                if not _DBG_ACT1024:
                    nc.scalar.activation(h2f[:, :, :], ps2[:, :], Tanh,
                                         bias=0.0, scale=1.0)
                else:
                    for gl in range(ngm):
                        nc.scalar.activation(
                            h2f[:, gl, :],
                            ps2[:, gl * 512:(gl + 1) * 512],
                            Tanh, bias=0.0, scale=1.0)
                # L3: w3-products alternate DVE/gpsimd (both otherwise
                # have slack), segmented 3D add-reduce on DVE
                prod = scpool.tile([128, ngm, 512], F16, tag="prod")
                for gl in range(ngm):
                    eng = nc.vector if gl % 2 == 0 else nc.gpsimd
                    eng.tensor_mul(prod[:, gl, :], h2f[:, gl, :],
                                   w3[:, :])
                nc.vector.tensor_reduce(
                    e_blk[:, groups[0]:groups[0] + ngm], prod[:, :, :],
                    mybir.AxisListType.X, add)
                h2cnt[bi] += ngm
                if h2cnt[bi] == ng:
                    nc.sync.dma_start(
                        e_out[:, boff // 128:boff // 128 + ng], e_blk[:, :])
                    del h1_of[bi], gs_of[bi], e_of[bi]

            # Software pipeline: L1(b) interleaved with L2(b-1) at
            # half-block granularity so PE work alternates between the
            # two 4-bank psum megas and never waits on an ACT drain.
            nblocks = len(blocks)
            for i in range(nblocks + 1):
                cur = i if i < nblocks else None
                prv = i - 1 if 0 <= i - 1 < nblocks else None
                if i == 1 and need_second:
                    # Deferred: the second expert's weights ride the gpsimd
                    # queue AFTER block 0's L3 products, so the early DVE
                    # reduce chain isn't stuck behind ~1MB of weight DMA.
                    emit_weight_dmas(second_ex, False)
                if cur is not None:
                    emit_l1(cur, 0)
                if prv is not None:
                    emit_l2(prv, 0)
                    emit_l2(prv, 1)
                if cur is not None:
                    emit_l1(cur, 1)
                if prv is not None:
                    emit_l2(prv, 2)
                    emit_l2(prv, 3)

    nc.compile()
    _PROGRAM_CACHE[key] = nc
    return nc


def kernel(**inputs) -> np.ndarray:
    Gs = np.ascontiguousarray(np.asarray(inputs["Gs"], dtype=np.float32))
    types = np.asarray(inputs["types"])
    mol_id = np.asarray(inputs["mol_id"])
    n_atoms = Gs.shape[0]

    idx = [np.flatnonzero(types == 0), np.flatnonzero(types != 0)]
    GRAN = 128
    n_a, n_b = (int(-(-len(ix) // (N_CORES * GRAN))) * GRAN for ix in idx)
    npc = n_a + n_b

    GsT = Gs.astype(np.float16).T  # [128, N] fp16 view

    wk = {}
    zb1 = zb2 = True
    for t, pre in (("a", "A"), ("b", "B")):
        wk[f"w1{t}"] = np.ascontiguousarray(
            np.asarray(inputs[f"W1_{pre}"], np.float32).astype(np.float16))
        wk[f"w2{t}"] = np.ascontiguousarray(
            np.asarray(inputs[f"W2_{pre}"], np.float32).astype(np.float16))
        w3col = np.asarray(inputs[f"W3_{pre}"], np.float32)[:, 0]
        wk[f"w3{t}"] = np.ascontiguousarray(
            np.broadcast_to(w3col.astype(np.float16), (128, HIDDEN)))
        b1 = np.asarray(inputs[f"b1_{pre}"], np.float32)
        b2 = np.asarray(inputs[f"b2_{pre}"], np.float32)
        if np.any(b1 != 0):
            zb1 = False
        if np.any(b2 != 0):
            zb2 = False
        wk[f"b1{t}"] = np.ascontiguousarray(b1.reshape(MCH, 128).T)
        wk[f"b2{t}"] = np.ascontiguousarray(np.broadcast_to(
            (b2 / 128.0).astype(np.float16), (128, HIDDEN)))
        wk[f"b3{t}"] = np.float32(
            np.asarray(inputs[f"b3_{pre}"], np.float32).reshape(())
            + np.asarray(inputs[f"off_{pre}"], np.float32).reshape(()))

    base = {f"w{j}{t}": wk[f"w{j}{t}"] for j in (1, 2, 3) for t in ("a", "b")}
    if not zb1:
        base.update({f"b1{t}": wk[f"b1{t}"] for t in ("a", "b")})
    if not zb2:
        base.update({f"b2{t}": wk[f"b2{t}"] for t in ("a", "b")})
        base["ones"] = np.ones((128, 128), np.float16)

    chunks = []  # per core: (a_indices, b_indices)
    in_maps = []
    for i in range(N_CORES):
        ca = idx[0][i * n_a:(i + 1) * n_a]
        cb = idx[1][i * n_b:(i + 1) * n_b]
        chunks.append((ca, cb))
        buf = np.zeros((NUM_GS, npc), np.float16)
        buf[:, :len(ca)] = GsT[:, ca]
        buf[:, n_a:n_a + len(cb)] = GsT[:, cb]
        in_maps.append({"gst": buf, **base})

    nc = _build_program(n_a, n_b, zb1, zb2)
    results = None
    for attempt in range(3):
        try:
            results = run_bass_kernel_spmd(
                nc, in_maps, list(range(N_CORES))).results
            break
        except Exception:
            # Transient NRT/device hiccups usually clear on retry.
            if attempt == 2:
                raise
            time.sleep(2.0)

    e = np.empty(n_atoms, np.float32)
    for i in range(N_CORES):
        r = np.asarray(results[i]["e_out"])  # [128, npc/128]
        flat = r.T.reshape(-1)
        ca, cb = chunks[i]
        e[ca] = flat[:len(ca)] + wk["b3a"]
        e[cb] = flat[n_a:n_a + len(cb)] + wk["b3b"]

    sums = np.bincount(mol_id, weights=e.astype(np.float64),
                       minlength=N_MOL)[:N_MOL]
    counts = np.bincount(mol_id, minlength=N_MOL)[:N_MOL]
    out = sums / np.maximum(counts, 1)
    return out.astype(np.float32)[:, None]
